# revision 2
# baseline (speedup 1.0000x reference)
"""GRU decoder kernel for 8 Trainium2 NeuronCores — v2 (bf16, full-batch ops).

Strategy (model-parallel over output features, weights resident in SBUF):
  - Each core owns a 256-row slice of H (per gate) and a 128-row slice of I.
  - Algebraic fusion: with W_comb = W_ih @ W_fc the r,z gates become a single
    K=2048 matmul with W_rz = (W_comb + W_hh)[r,z rows]; i_n uses W_comb[n],
    h_n uses W_hh[n].  One bf16 AllGather of h per step.
  - All matmul operands are bf16 (PSUM accumulation stays f32); measured
    rel err ~3e-3 vs the f32 reference (gate is 2e-2).
  - Full-batch N=512 ops throughout: half the instruction count of the
    two-half variant (per-instruction overhead dominates in this env).
  - Fused vector ops: (a_hn + b_hn) * r in one scalar_tensor_tensor.
  - fc matmuls for y[t-1] run from the already-gathered h_t at the START of
    step t, so the post-AllGather critical path is only the gather-in DMA.
"""

import numpy as np
import ml_dtypes

import concourse.mybir as mybir
import concourse.tile as tile
from concourse import bacc
from concourse.bass_utils import run_bass_kernel_spmd

F32 = mybir.dt.float32
BF16 = mybir.dt.bfloat16
AF = mybir.ActivationFunctionType
ALU = mybir.AluOpType

G = 8          # cores
B = 512        # batch
I = 1024       # input/output feature dim
H = 2048       # hidden dim
HL = H // G    # 256 hidden rows per core (per gate)
XL = I // G    # 128 fc output rows per core
KH = H // 128  # 16 k-tiles over H
KI = I // 128  # 8 k-tiles over I

NPBF16 = ml_dtypes.bfloat16


def build(T: int):
    """Emit the SPMD program for T timesteps."""
    nc = bacc.Bacc("TRN2", target_bir_lowering=False, debug=False, num_devices=G)
    dp = nc.declare_dram_parameter

    w_rz = dp("w_rz", [H, 512], BF16, isOutput=False)    # (Wcomb+Whh)[r|z].T
    w_cn = dp("w_cn", [H, 256], BF16, isOutput=False)    # Wcomb[n].T
    w_hn = dp("w_hn", [H, 256], BF16, isOutput=False)    # Whh[n].T
    w_fc = dp("w_fc", [H, 128], BF16, isOutput=False)    # Wfc[own].T
    w_ih0 = dp("w_ih0", [I, 768], BF16, isOutput=False)  # Wih[r|z|n].T (t=0)
    w_hh0 = dp("w_hh0", [H, 512], BF16, isOutput=False)  # Whh[r|z].T   (t=0)
    x0 = dp("x0", [I, B], BF16, isOutput=False)          # inputs.T
    h0 = dp("h0", [H, B], BF16, isOutput=False)          # h_0.T
    h0_own = dp("h0_own", [HL, B], BF16, isOutput=False)  # own h_0 rows
    b_rz = dp("b_rz", [128, 4], F32, isOutput=False)     # r0 r1 z0 z1, t>=1
    b_rz0 = dp("b_rz0", [128, 4], F32, isOutput=False)   # t=0
    b_in = dp("b_in", [128, 2], F32, isOutput=False)
    b_in0 = dp("b_in0", [128, 2], F32, isOutput=False)
    b_hn = dp("b_hn", [128, 2], F32, isOutput=False)
    b_fc = dp("b_fc", [128, 1], F32, isOutput=False)
    y = dp("y", [T, 128, B], F32, isOutput=True)

    hstage = [nc.dram_tensor(f"hstage{s}", [HL, B], BF16) for s in (0, 1)]
    hgath = [
        nc.dram_tensor(f"hgath{s}", [H, B], BF16, addr_space="Shared")
        for s in (0, 1)
    ]

    with tile.TileContext(nc) as tc:
        with (
            tc.tile_pool(name="weights", bufs=1) as wp,
            tc.tile_pool(name="state", bufs=1) as stp,
            tc.tile_pool(name="scratch", bufs=2) as scr,
            tc.tile_pool(name="w0pool", bufs=3) as w0p,
            tc.tile_pool(name="psum", bufs=8, space="PSUM") as psp,
        ):
            # ---- persistent weights -------------------------------------
            w_rz_sb = wp.tile([128, KH, 512], BF16, tag="w_rz")
            w_cn_sb = wp.tile([128, KH, 256], BF16, tag="w_cn")
            w_hn_sb = wp.tile([128, KH, 256], BF16, tag="w_hn")
            w_fc_sb = wp.tile([128, KH, 128], BF16, tag="w_fc")
            for k0 in range(0, KH, 4):
                sl = slice(k0, k0 + 4)
                nc.sync.dma_start(
                    w_rz_sb[:, sl, :],
                    w_rz[:].rearrange("(k p) m -> p k m", p=128)[:, sl, :],
                )
                nc.sync.dma_start(
                    w_cn_sb[:, sl, :],
                    w_cn[:].rearrange("(k p) m -> p k m", p=128)[:, sl, :],
                )
                nc.sync.dma_start(
                    w_hn_sb[:, sl, :],
                    w_hn[:].rearrange("(k p) m -> p k m", p=128)[:, sl, :],
                )
            nc.sync.dma_start(
                w_fc_sb[:],
                w_fc[:].rearrange("(k p) m -> p k m", p=128),
            )

            # ---- biases --------------------------------------------------
            def bias_tile(param, ncols, tag):
                t = wp.tile([128, ncols], F32, tag=tag)
                nc.sync.dma_start(t[:], param[:])
                return t

            b_rz_sb = bias_tile(b_rz, 4, "b_rz")
            b_rz0_sb = bias_tile(b_rz0, 4, "b_rz0")
            b_in_sb = bias_tile(b_in, 2, "b_in")
            b_in0_sb = bias_tile(b_in0, 2, "b_in0")
            b_hn_sb = bias_tile(b_hn, 2, "b_hn")
            b_fc_sb = bias_tile(b_fc, 1, "b_fc")

            # ---- state: gathered h (ping-pong), own h slice -------------
            ht_sb = [
                stp.tile([128, KH, B], BF16, tag=f"ht{pp}", name=f"ht{pp}")
                for pp in (0, 1)
            ]
            h_own = [
                stp.tile([128, 2, B], BF16, tag=f"ho{pp}", name=f"ho{pp}")
                for pp in (0, 1)
            ]
            for k0 in range(0, KH, 4):
                sl = slice(k0, k0 + 4)
                nc.sync.dma_start(
                    ht_sb[0][:, sl, :],
                    h0[:].rearrange("(k p) n -> p k n", p=128)[:, sl, :],
                )
            nc.sync.dma_start(
                h_own[0][:],
                h0_own[:].rearrange("(j p) n -> p j n", p=128),
            )
            x0_sb = stp.tile([128, KI, B], BF16, tag="x0")
            nc.sync.dma_start(
                x0_sb[:], x0[:].rearrange("(k p) n -> p k n", p=128)
            )

            # ---- time loop ----------------------------------------------
            for t in range(T):
                cur, nxt = t % 2, 1 - (t % 2)
                ht_c = ht_sb[cur]
                brz = b_rz_sb if t > 0 else b_rz0_sb
                bin_ = b_in_sb if t > 0 else b_in0_sb

                # fc output y[t-1] = h_t @ Wfc.T + b_fc (h_t already here)
                if t > 0:
                    ps_fc = psp.tile([128, B], F32, tag="ps", name="ps_fc")
                    for k in range(KH):
                        nc.tensor.matmul(
                            ps_fc[:], w_fc_sb[:, k, :], ht_c[:, k, :],
                            start=(k == 0), stop=(k == KH - 1),
                        )
                    y_sb = scr.tile([128, B], F32, tag="y")
                    nc.scalar.activation(
                        y_sb[:], ps_fc[:], AF.Identity, bias=b_fc_sb[:, 0:1]
                    )
                    nc.sync.dma_start(y[t - 1, :, :], y_sb[:])

                ps_r = [None, None]
                ps_z = [None, None]
                ps_in = [None, None]
                ps_hn = [None, None]
                if t == 0:
                    for j in (0, 1):
                        ps_r[j] = psp.tile([128, B], F32, tag="ps", name="ps_r")
                        ps_z[j] = psp.tile([128, B], F32, tag="ps", name="ps_z")
                        ps_in[j] = psp.tile([128, B], F32, tag="ps", name="ps_in")
                        ps_hn[j] = psp.tile([128, B], F32, tag="ps", name="ps_hn")
                    wih_r = w_ih0[:].rearrange("(k p) m -> p k m", p=128)
                    for k in range(KI):
                        wt = w0p.tile([128, 768], BF16, tag="w0ih")
                        nc.sync.dma_start(wt[:], wih_r[:, k, :])
                        for j in (0, 1):
                            nc.tensor.matmul(
                                ps_r[j][:], wt[:, j * 128:(j + 1) * 128],
                                x0_sb[:, k, :], start=(k == 0), stop=False,
                            )
                            nc.tensor.matmul(
                                ps_z[j][:], wt[:, 256 + j * 128:384 + j * 128],
                                x0_sb[:, k, :], start=(k == 0), stop=False,
                            )
                            nc.tensor.matmul(
                                ps_in[j][:], wt[:, 512 + j * 128:640 + j * 128],
                                x0_sb[:, k, :],
                                start=(k == 0), stop=(k == KI - 1),
                            )
                    whh_r = w_hh0[:].rearrange("(k p) m -> p k m", p=128)
                    for k in range(KH):
                        wt = w0p.tile([128, 512], BF16, tag="w0hh")
                        nc.sync.dma_start(wt[:], whh_r[:, k, :])
                        for j in (0, 1):
                            nc.tensor.matmul(
                                ps_r[j][:], wt[:, j * 128:(j + 1) * 128],
                                ht_c[:, k, :], start=False, stop=(k == KH - 1),
                            )
                            nc.tensor.matmul(
                                ps_z[j][:], wt[:, 256 + j * 128:384 + j * 128],
                                ht_c[:, k, :], start=False, stop=(k == KH - 1),
                            )
                    for j in (0, 1):
                        for k in range(KH):
                            nc.tensor.matmul(
                                ps_hn[j][:],
                                w_hn_sb[:, k, j * 128:(j + 1) * 128],
                                ht_c[:, k, :],
                                start=(k == 0), stop=(k == KH - 1),
                            )
                else:
                    for j in (0, 1):
                        ps_r[j] = psp.tile([128, B], F32, tag="ps", name="ps_r")
                        for k in range(KH):
                            nc.tensor.matmul(
                                ps_r[j][:],
                                w_rz_sb[:, k, j * 128:(j + 1) * 128],
                                ht_c[:, k, :],
                                start=(k == 0), stop=(k == KH - 1),
                            )
                        ps_z[j] = psp.tile([128, B], F32, tag="ps", name="ps_z")
                        for k in range(KH):
                            nc.tensor.matmul(
                                ps_z[j][:],
                                w_rz_sb[:, k, 256 + j * 128:384 + j * 128],
                                ht_c[:, k, :],
                                start=(k == 0), stop=(k == KH - 1),
                            )
                        ps_in[j] = psp.tile([128, B], F32, tag="ps", name="ps_in")
                        for k in range(KH):
                            nc.tensor.matmul(
                                ps_in[j][:],
                                w_cn_sb[:, k, j * 128:(j + 1) * 128],
                                ht_c[:, k, :],
                                start=(k == 0), stop=(k == KH - 1),
                            )
                        ps_hn[j] = psp.tile([128, B], F32, tag="ps", name="ps_hn")
                        for k in range(KH):
                            nc.tensor.matmul(
                                ps_hn[j][:],
                                w_hn_sb[:, k, j * 128:(j + 1) * 128],
                                ht_c[:, k, :],
                                start=(k == 0), stop=(k == KH - 1),
                            )

                # nonlinearity chain per j-tile (N=512)
                for j in (0, 1):
                    r_t = scr.tile([128, B], F32, tag="r")
                    nc.scalar.activation(
                        r_t[:], ps_r[j][:], AF.Sigmoid, bias=brz[:, j:j + 1]
                    )
                    z_t = scr.tile([128, B], F32, tag="z")
                    nc.scalar.activation(
                        z_t[:], ps_z[j][:], AF.Sigmoid, bias=brz[:, 2 + j:3 + j]
                    )
                    # m1 = (a_hn + b_hn) * r
                    m1 = scr.tile([128, B], F32, tag="m1")
                    nc.vector.scalar_tensor_tensor(
                        m1[:], ps_hn[j][:], b_hn_sb[:, j:j + 1], r_t[:],
                        ALU.add, ALU.mult,
                    )
                    s1 = scr.tile([128, B], F32, tag="s1")
                    nc.vector.tensor_add(s1[:], ps_in[j][:], m1[:])
                    n_t = scr.tile([128, B], F32, tag="n")
                    nc.scalar.activation(
                        n_t[:], s1[:], AF.Tanh, bias=bin_[:, j:j + 1]
                    )
                    d_t = scr.tile([128, B], F32, tag="d")
                    nc.vector.tensor_sub(
                        d_t[:], h_own[cur][:, j, :], n_t[:]
                    )
                    zd = scr.tile([128, B], F32, tag="zd")
                    nc.vector.tensor_mul(zd[:], z_t[:], d_t[:])
                    # bf16 write of the new own h slice
                    nc.vector.tensor_add(
                        h_own[nxt][:, j, :], n_t[:], zd[:]
                    )
                    nc.sync.dma_start(
                        hstage[cur][j * 128:(j + 1) * 128, :],
                        h_own[nxt][:, j, :],
                    )

                # ---- one all-gather per step ----------------------------
                nc.gpsimd.collective_compute(
                    "AllGather",
                    mybir.AluOpType.bypass,
                    replica_groups=[list(range(G))],
                    ins=[hstage[cur][:]],
                    outs=[hgath[cur][:]],
                )
                gat = hgath[cur][:].rearrange("(k p) n -> p k n", p=128)
                for k0 in range(0, KH, 4):
                    sl = slice(k0, k0 + 4)
                    nc.sync.dma_start(ht_sb[nxt][:, sl, :], gat[:, sl, :])

            # final fc: y[T-1] = h_T @ Wfc.T + b_fc
            ht_f = ht_sb[T % 2]
            ps_fc = psp.tile([128, B], F32, tag="ps", name="ps_fcf")
            for k in range(KH):
                nc.tensor.matmul(
                    ps_fc[:], w_fc_sb[:, k, :], ht_f[:, k, :],
                    start=(k == 0), stop=(k == KH - 1),
                )
            y_sb = scr.tile([128, B], F32, tag="y")
            nc.scalar.activation(
                y_sb[:], ps_fc[:], AF.Identity, bias=b_fc_sb[:, 0:1]
            )
            nc.sync.dma_start(y[T - 1, :, :], y_sb[:])

    nc.compile()
    return nc


def prep_in_maps(inputs, h_0, W_ih, W_hh, b_ih, b_hh, W_fc, b_fc):
    """Host-side sharding/layout prep. Returns per-core in_maps."""
    W_ih64 = np.asarray(W_ih, np.float64)
    W_hh64 = np.asarray(W_hh, np.float64)
    W_fc64 = np.asarray(W_fc, np.float64)
    b_ih = np.asarray(b_ih, np.float32)
    b_hh = np.asarray(b_hh, np.float32)
    b_fc32 = np.asarray(b_fc, np.float32)

    Wc = W_ih64 @ W_fc64                       # [3H, H]
    bias_comb = W_ih64 @ np.asarray(b_fc, np.float64)  # [3H]

    x0_t = np.asarray(inputs, np.float32).T.astype(NPBF16)
    h0_t = np.asarray(h_0, np.float32).T.astype(NPBF16)

    in_maps = []
    for c in range(G):
        rs = np.arange(HL * c, HL * (c + 1))
        idx_rz = np.concatenate([rs, H + rs])
        idx_n = 2 * H + rs
        idx_rzn = np.concatenate([idx_rz, idx_n])
        xs = slice(XL * c, XL * (c + 1))

        w_rz_c = (Wc[idx_rz] + W_hh64[idx_rz]).T.astype(NPBF16)
        w_cn_c = Wc[idx_n].T.astype(NPBF16)
        w_hn_c = W_hh64[idx_n].T.astype(NPBF16)
        w_fc_c = W_fc64[xs].T.astype(NPBF16)
        w_ih0_c = W_ih64[idx_rzn].T.astype(NPBF16)
        w_hh0_c = W_hh64[idx_rz].T.astype(NPBF16)

        b_rz_c = (b_ih[idx_rz].astype(np.float64)
                  + b_hh[idx_rz] + bias_comb[idx_rz]).astype(np.float32)
        b_rz0_c = b_ih[idx_rz] + b_hh[idx_rz]
        b_in_c = (b_ih[idx_n].astype(np.float64)
                  + bias_comb[idx_n]).astype(np.float32)
        b_in0_c = b_ih[idx_n]
        b_hn_c = b_hh[idx_n]

        in_maps.append({
            "w_rz": np.ascontiguousarray(w_rz_c),
            "w_cn": np.ascontiguousarray(w_cn_c),
            "w_hn": np.ascontiguousarray(w_hn_c),
            "w_fc": np.ascontiguousarray(w_fc_c),
            "w_ih0": np.ascontiguousarray(w_ih0_c),
            "w_hh0": np.ascontiguousarray(w_hh0_c),
            "x0": x0_t,
            "h0": h0_t,
            "h0_own": np.ascontiguousarray(h0_t[HL * c:HL * (c + 1)]),
            "b_rz": np.ascontiguousarray(b_rz_c.reshape(4, 128).T),
            "b_rz0": np.ascontiguousarray(b_rz0_c.reshape(4, 128).T),
            "b_in": np.ascontiguousarray(b_in_c.reshape(2, 128).T),
            "b_in0": np.ascontiguousarray(b_in0_c.reshape(2, 128).T),
            "b_hn": np.ascontiguousarray(b_hn_c.reshape(2, 128).T),
            "b_fc": np.ascontiguousarray(b_fc32[xs].reshape(1, 128).T),
        })
    return in_maps


def assemble_output(results, T: int) -> np.ndarray:
    """Per-core y [T, 128, B] (features x batch) -> [B, T, I], time-reversed."""
    out = np.empty((B, T, I), np.float32)
    for c, res in enumerate(results):
        yc = res["y"]                      # [T, 128, B]
        out[:, :, XL * c:XL * (c + 1)] = yc[::-1].transpose(2, 0, 1)
    return out


_NC_CACHE: dict = {}


def _get_nc(T: int):
    if T not in _NC_CACHE:
        _NC_CACHE[T] = build(T)
    return _NC_CACHE[T]


def kernel(inputs, h_0, W_ih, W_hh, b_ih, b_hh, W_fc, b_fc, seq_len):
    T = int(seq_len)
    nc = _get_nc(T)
    in_maps = prep_in_maps(inputs, h_0, W_ih, W_hh, b_ih, b_hh, W_fc, b_fc)
    res = run_bass_kernel_spmd(nc, in_maps, list(range(G)))
    return assemble_output(res.results, T)


# revision 4
# speedup vs baseline: 1.0940x; 1.0940x over previous
"""GRU decoder kernel for 8 Trainium2 NeuronCores — v2 (bf16, full-batch ops).

Strategy (model-parallel over output features, weights resident in SBUF):
  - Each core owns a 256-row slice of H (per gate) and a 128-row slice of I.
  - Algebraic fusion: with W_comb = W_ih @ W_fc the r,z gates become a single
    K=2048 matmul with W_rz = (W_comb + W_hh)[r,z rows]; i_n uses W_comb[n],
    h_n uses W_hh[n].  One bf16 AllGather of h per step.
  - All matmul operands are bf16 (PSUM accumulation stays f32); measured
    rel err ~3e-3 vs the f32 reference (gate is 2e-2).
  - Full-batch N=512 ops throughout: half the instruction count of the
    two-half variant (per-instruction overhead dominates in this env).
  - Fused vector ops: (a_hn + b_hn) * r in one scalar_tensor_tensor.
  - fc matmuls for y[t-1] run from the already-gathered h_t at the START of
    step t, so the post-AllGather critical path is only the gather-in DMA.
"""

import numpy as np
import ml_dtypes

import concourse.mybir as mybir
import concourse.tile as tile
from concourse import bacc
from concourse.bass_utils import run_bass_kernel_spmd

F32 = mybir.dt.float32
BF16 = mybir.dt.bfloat16
AF = mybir.ActivationFunctionType
ALU = mybir.AluOpType

G = 8          # cores
B = 512        # batch
I = 1024       # input/output feature dim
H = 2048       # hidden dim
HL = H // G    # 256 hidden rows per core (per gate)
XL = I // G    # 128 fc output rows per core
KH = H // 128  # 16 k-tiles over H
KI = I // 128  # 8 k-tiles over I

NPBF16 = ml_dtypes.bfloat16


def build(T: int):
    """Emit the SPMD program for T timesteps."""
    nc = bacc.Bacc("TRN2", target_bir_lowering=False, debug=False, num_devices=G)
    dp = nc.declare_dram_parameter

    w_rz = dp("w_rz", [H, 512], BF16, isOutput=False)    # (Wcomb+Whh)[r|z].T
    w_cn = dp("w_cn", [H, 256], BF16, isOutput=False)    # Wcomb[n].T
    w_hn = dp("w_hn", [H, 256], BF16, isOutput=False)    # Whh[n].T
    w_fc = dp("w_fc", [H, 128], BF16, isOutput=False)    # Wfc[own].T
    w_ih0 = dp("w_ih0", [I, 768], BF16, isOutput=False)  # Wih[r|z|n].T (t=0)
    w_hh0 = dp("w_hh0", [H, 512], BF16, isOutput=False)  # Whh[r|z].T   (t=0)
    x0 = dp("x0", [I, B], BF16, isOutput=False)          # inputs.T
    h0 = dp("h0", [H, B], BF16, isOutput=False)          # h_0.T
    h0_own = dp("h0_own", [HL, B], BF16, isOutput=False)  # own h_0 rows
    b_rz = dp("b_rz", [128, 4], F32, isOutput=False)     # r0 r1 z0 z1, t>=1
    b_rz0 = dp("b_rz0", [128, 4], F32, isOutput=False)   # t=0
    b_in = dp("b_in", [128, 2], F32, isOutput=False)
    b_in0 = dp("b_in0", [128, 2], F32, isOutput=False)
    b_hn = dp("b_hn", [128, 2], F32, isOutput=False)
    b_fc = dp("b_fc", [128, 1], F32, isOutput=False)
    y = dp("y", [T, 128, B], F32, isOutput=True)

    hstage = [nc.dram_tensor(f"hstage{s}", [HL, B], BF16) for s in (0, 1)]
    hgath = [
        nc.dram_tensor(f"hgath{s}", [H, B], BF16, addr_space="Shared")
        for s in (0, 1)
    ]

    with tile.TileContext(nc) as tc:
        with (
            tc.tile_pool(name="weights", bufs=1) as wp,
            tc.tile_pool(name="state", bufs=1) as stp,
            tc.tile_pool(name="scratch", bufs=2) as scr,
            tc.tile_pool(name="w0pool", bufs=3) as w0p,
            tc.tile_pool(name="psum", bufs=8, space="PSUM") as psp,
        ):
            # ---- persistent weights -------------------------------------
            w_rz_sb = wp.tile([128, KH, 512], BF16, tag="w_rz")
            w_cn_sb = wp.tile([128, KH, 256], BF16, tag="w_cn")
            w_hn_sb = wp.tile([128, KH, 256], BF16, tag="w_hn")
            w_fc_sb = wp.tile([128, KH, 128], BF16, tag="w_fc")
            for k0 in range(0, KH, 4):
                sl = slice(k0, k0 + 4)
                nc.sync.dma_start(
                    w_rz_sb[:, sl, :],
                    w_rz[:].rearrange("(k p) m -> p k m", p=128)[:, sl, :],
                )
                nc.sync.dma_start(
                    w_cn_sb[:, sl, :],
                    w_cn[:].rearrange("(k p) m -> p k m", p=128)[:, sl, :],
                )
                nc.sync.dma_start(
                    w_hn_sb[:, sl, :],
                    w_hn[:].rearrange("(k p) m -> p k m", p=128)[:, sl, :],
                )
            nc.sync.dma_start(
                w_fc_sb[:],
                w_fc[:].rearrange("(k p) m -> p k m", p=128),
            )

            # ---- biases --------------------------------------------------
            def bias_tile(param, ncols, tag):
                t = wp.tile([128, ncols], F32, tag=tag)
                nc.sync.dma_start(t[:], param[:])
                return t

            b_rz_sb = bias_tile(b_rz, 4, "b_rz")
            b_rz0_sb = bias_tile(b_rz0, 4, "b_rz0")
            b_in_sb = bias_tile(b_in, 2, "b_in")
            b_in0_sb = bias_tile(b_in0, 2, "b_in0")
            b_hn_sb = bias_tile(b_hn, 2, "b_hn")
            b_fc_sb = bias_tile(b_fc, 1, "b_fc")

            # ---- state: gathered h (ping-pong), own h slice -------------
            ht_sb = [
                stp.tile([128, KH, B], BF16, tag=f"ht{pp}", name=f"ht{pp}")
                for pp in (0, 1)
            ]
            h_own = [
                stp.tile([128, 2, B], BF16, tag=f"ho{pp}", name=f"ho{pp}")
                for pp in (0, 1)
            ]
            for k0 in range(0, KH, 4):
                sl = slice(k0, k0 + 4)
                nc.sync.dma_start(
                    ht_sb[0][:, sl, :],
                    h0[:].rearrange("(k p) n -> p k n", p=128)[:, sl, :],
                )
            nc.sync.dma_start(
                h_own[0][:],
                h0_own[:].rearrange("(j p) n -> p j n", p=128),
            )
            x0_sb = stp.tile([128, KI, B], BF16, tag="x0")
            nc.sync.dma_start(
                x0_sb[:], x0[:].rearrange("(k p) n -> p k n", p=128)
            )

            # ---- time loop ----------------------------------------------
            for t in range(T):
                cur, nxt = t % 2, 1 - (t % 2)
                ht_c = ht_sb[cur]
                brz = b_rz_sb if t > 0 else b_rz0_sb
                bin_ = b_in_sb if t > 0 else b_in0_sb

                # fc output y[t-1] = h_t @ Wfc.T + b_fc (h_t already here)
                if t > 0:
                    ps_fc = psp.tile([128, B], F32, tag="ps", name="ps_fc")
                    for k in range(KH):
                        nc.tensor.matmul(
                            ps_fc[:], w_fc_sb[:, k, :], ht_c[:, k, :],
                            start=(k == 0), stop=(k == KH - 1),
                        )
                    y_sb = scr.tile([128, B], F32, tag="y")
                    nc.scalar.activation(
                        y_sb[:], ps_fc[:], AF.Identity, bias=b_fc_sb[:, 0:1]
                    )
                    nc.sync.dma_start(y[t - 1, :, :], y_sb[:])

                ps_r = [None, None]
                ps_z = [None, None]
                ps_in = [None, None]
                ps_hn = [None, None]
                if t == 0:
                    for j in (0, 1):
                        ps_r[j] = psp.tile([128, B], F32, tag="ps", name="ps_r")
                        ps_z[j] = psp.tile([128, B], F32, tag="ps", name="ps_z")
                        ps_in[j] = psp.tile([128, B], F32, tag="ps", name="ps_in")
                        ps_hn[j] = psp.tile([128, B], F32, tag="ps", name="ps_hn")
                    wih_r = w_ih0[:].rearrange("(k p) m -> p k m", p=128)
                    for k in range(KI):
                        wt = w0p.tile([128, 768], BF16, tag="w0ih")
                        nc.sync.dma_start(wt[:], wih_r[:, k, :])
                        for j in (0, 1):
                            nc.tensor.matmul(
                                ps_r[j][:], wt[:, j * 128:(j + 1) * 128],
                                x0_sb[:, k, :], start=(k == 0), stop=False,
                            )
                            nc.tensor.matmul(
                                ps_z[j][:], wt[:, 256 + j * 128:384 + j * 128],
                                x0_sb[:, k, :], start=(k == 0), stop=False,
                            )
                            nc.tensor.matmul(
                                ps_in[j][:], wt[:, 512 + j * 128:640 + j * 128],
                                x0_sb[:, k, :],
                                start=(k == 0), stop=(k == KI - 1),
                            )
                    whh_r = w_hh0[:].rearrange("(k p) m -> p k m", p=128)
                    for k in range(KH):
                        wt = w0p.tile([128, 512], BF16, tag="w0hh")
                        nc.sync.dma_start(wt[:], whh_r[:, k, :])
                        for j in (0, 1):
                            nc.tensor.matmul(
                                ps_r[j][:], wt[:, j * 128:(j + 1) * 128],
                                ht_c[:, k, :], start=False, stop=(k == KH - 1),
                            )
                            nc.tensor.matmul(
                                ps_z[j][:], wt[:, 256 + j * 128:384 + j * 128],
                                ht_c[:, k, :], start=False, stop=(k == KH - 1),
                            )
                    for j in (0, 1):
                        for k in range(KH):
                            nc.tensor.matmul(
                                ps_hn[j][:],
                                w_hn_sb[:, k, j * 128:(j + 1) * 128],
                                ht_c[:, k, :],
                                start=(k == 0), stop=(k == KH - 1),
                            )
                else:
                    # chain order (r, hn, in, z): the tail's first fused op
                    # needs r and hn, its last needs z — starts ~32 matmuls
                    # earlier than (r, z, in, hn) order.
                    for j in (0, 1):
                        ps_r[j] = psp.tile([128, B], F32, tag="ps", name="ps_r")
                        for k in range(KH):
                            nc.tensor.matmul(
                                ps_r[j][:],
                                w_rz_sb[:, k, j * 128:(j + 1) * 128],
                                ht_c[:, k, :],
                                start=(k == 0), stop=(k == KH - 1),
                            )
                        ps_hn[j] = psp.tile([128, B], F32, tag="ps", name="ps_hn")
                        for k in range(KH):
                            nc.tensor.matmul(
                                ps_hn[j][:],
                                w_hn_sb[:, k, j * 128:(j + 1) * 128],
                                ht_c[:, k, :],
                                start=(k == 0), stop=(k == KH - 1),
                            )
                        ps_in[j] = psp.tile([128, B], F32, tag="ps", name="ps_in")
                        for k in range(KH):
                            nc.tensor.matmul(
                                ps_in[j][:],
                                w_cn_sb[:, k, j * 128:(j + 1) * 128],
                                ht_c[:, k, :],
                                start=(k == 0), stop=(k == KH - 1),
                            )
                        ps_z[j] = psp.tile([128, B], F32, tag="ps", name="ps_z")
                        for k in range(KH):
                            nc.tensor.matmul(
                                ps_z[j][:],
                                w_rz_sb[:, k, 256 + j * 128:384 + j * 128],
                                ht_c[:, k, :],
                                start=(k == 0), stop=(k == KH - 1),
                            )

                # nonlinearity chain per j-tile (N=512)
                for j in (0, 1):
                    r_t = scr.tile([128, B], F32, tag="r")
                    nc.scalar.activation(
                        r_t[:], ps_r[j][:], AF.Sigmoid, bias=brz[:, j:j + 1]
                    )
                    z_t = scr.tile([128, B], F32, tag="z")
                    nc.scalar.activation(
                        z_t[:], ps_z[j][:], AF.Sigmoid, bias=brz[:, 2 + j:3 + j]
                    )
                    # m1 = (a_hn + b_hn) * r
                    m1 = scr.tile([128, B], F32, tag="m1")
                    nc.vector.scalar_tensor_tensor(
                        m1[:], ps_hn[j][:], b_hn_sb[:, j:j + 1], r_t[:],
                        ALU.add, ALU.mult,
                    )
                    s1 = scr.tile([128, B], F32, tag="s1")
                    nc.vector.tensor_add(s1[:], ps_in[j][:], m1[:])
                    n_t = scr.tile([128, B], F32, tag="n")
                    nc.scalar.activation(
                        n_t[:], s1[:], AF.Tanh, bias=bin_[:, j:j + 1]
                    )
                    d_t = scr.tile([128, B], F32, tag="d")
                    nc.vector.tensor_sub(
                        d_t[:], h_own[cur][:, j, :], n_t[:]
                    )
                    zd = scr.tile([128, B], F32, tag="zd")
                    nc.vector.tensor_mul(zd[:], z_t[:], d_t[:])
                    # bf16 write of the new own h slice
                    nc.vector.tensor_add(
                        h_own[nxt][:, j, :], n_t[:], zd[:]
                    )

                # single stage-out DMA for both j slices
                nc.sync.dma_start(
                    hstage[cur][:].rearrange("(j p) n -> p j n", p=128),
                    h_own[nxt][:],
                )

                # ---- one all-gather per step ----------------------------
                nc.gpsimd.collective_compute(
                    "AllGather",
                    mybir.AluOpType.bypass,
                    replica_groups=[list(range(G))],
                    ins=[hstage[cur][:]],
                    outs=[hgath[cur][:]],
                )
                gat = hgath[cur][:].rearrange("(k p) n -> p k n", p=128)
                for k0 in range(0, KH, 4):
                    sl = slice(k0, k0 + 4)
                    nc.sync.dma_start(ht_sb[nxt][:, sl, :], gat[:, sl, :])

            # final fc: y[T-1] = h_T @ Wfc.T + b_fc
            ht_f = ht_sb[T % 2]
            ps_fc = psp.tile([128, B], F32, tag="ps", name="ps_fcf")
            for k in range(KH):
                nc.tensor.matmul(
                    ps_fc[:], w_fc_sb[:, k, :], ht_f[:, k, :],
                    start=(k == 0), stop=(k == KH - 1),
                )
            y_sb = scr.tile([128, B], F32, tag="y")
            nc.scalar.activation(
                y_sb[:], ps_fc[:], AF.Identity, bias=b_fc_sb[:, 0:1]
            )
            nc.sync.dma_start(y[T - 1, :, :], y_sb[:])

    nc.compile()
    return nc


def prep_in_maps(inputs, h_0, W_ih, W_hh, b_ih, b_hh, W_fc, b_fc):
    """Host-side sharding/layout prep. Returns per-core in_maps."""
    W_ih64 = np.asarray(W_ih, np.float64)
    W_hh64 = np.asarray(W_hh, np.float64)
    W_fc64 = np.asarray(W_fc, np.float64)
    b_ih = np.asarray(b_ih, np.float32)
    b_hh = np.asarray(b_hh, np.float32)
    b_fc32 = np.asarray(b_fc, np.float32)

    Wc = W_ih64 @ W_fc64                       # [3H, H]
    bias_comb = W_ih64 @ np.asarray(b_fc, np.float64)  # [3H]

    x0_t = np.asarray(inputs, np.float32).T.astype(NPBF16)
    h0_t = np.asarray(h_0, np.float32).T.astype(NPBF16)

    in_maps = []
    for c in range(G):
        rs = np.arange(HL * c, HL * (c + 1))
        idx_rz = np.concatenate([rs, H + rs])
        idx_n = 2 * H + rs
        idx_rzn = np.concatenate([idx_rz, idx_n])
        xs = slice(XL * c, XL * (c + 1))

        w_rz_c = (Wc[idx_rz] + W_hh64[idx_rz]).T.astype(NPBF16)
        w_cn_c = Wc[idx_n].T.astype(NPBF16)
        w_hn_c = W_hh64[idx_n].T.astype(NPBF16)
        w_fc_c = W_fc64[xs].T.astype(NPBF16)
        w_ih0_c = W_ih64[idx_rzn].T.astype(NPBF16)
        w_hh0_c = W_hh64[idx_rz].T.astype(NPBF16)

        b_rz_c = (b_ih[idx_rz].astype(np.float64)
                  + b_hh[idx_rz] + bias_comb[idx_rz]).astype(np.float32)
        b_rz0_c = b_ih[idx_rz] + b_hh[idx_rz]
        b_in_c = (b_ih[idx_n].astype(np.float64)
                  + bias_comb[idx_n]).astype(np.float32)
        b_in0_c = b_ih[idx_n]
        b_hn_c = b_hh[idx_n]

        in_maps.append({
            "w_rz": np.ascontiguousarray(w_rz_c),
            "w_cn": np.ascontiguousarray(w_cn_c),
            "w_hn": np.ascontiguousarray(w_hn_c),
            "w_fc": np.ascontiguousarray(w_fc_c),
            "w_ih0": np.ascontiguousarray(w_ih0_c),
            "w_hh0": np.ascontiguousarray(w_hh0_c),
            "x0": x0_t,
            "h0": h0_t,
            "h0_own": np.ascontiguousarray(h0_t[HL * c:HL * (c + 1)]),
            "b_rz": np.ascontiguousarray(b_rz_c.reshape(4, 128).T),
            "b_rz0": np.ascontiguousarray(b_rz0_c.reshape(4, 128).T),
            "b_in": np.ascontiguousarray(b_in_c.reshape(2, 128).T),
            "b_in0": np.ascontiguousarray(b_in0_c.reshape(2, 128).T),
            "b_hn": np.ascontiguousarray(b_hn_c.reshape(2, 128).T),
            "b_fc": np.ascontiguousarray(b_fc32[xs].reshape(1, 128).T),
        })
    return in_maps


def assemble_output(results, T: int) -> np.ndarray:
    """Per-core y [T, 128, B] (features x batch) -> [B, T, I], time-reversed."""
    out = np.empty((B, T, I), np.float32)
    for c, res in enumerate(results):
        yc = res["y"]                      # [T, 128, B]
        out[:, :, XL * c:XL * (c + 1)] = yc[::-1].transpose(2, 0, 1)
    return out


_NC_CACHE: dict = {}


def _get_nc(T: int):
    if T not in _NC_CACHE:
        _NC_CACHE[T] = build(T)
    return _NC_CACHE[T]


def kernel(inputs, h_0, W_ih, W_hh, b_ih, b_hh, W_fc, b_fc, seq_len):
    T = int(seq_len)
    nc = _get_nc(T)
    in_maps = prep_in_maps(inputs, h_0, W_ih, W_hh, b_ih, b_hh, W_fc, b_fc)
    res = run_bass_kernel_spmd(nc, in_maps, list(range(G)))
    return assemble_output(res.results, T)
